# revision 26
# baseline (speedup 1.0000x reference)
"""Trainium2 Bass kernel for nn_CodeformerLM (masked embedding -> W_dec -> logits).

The reference computation provably reduces to (see analysis in test.py):
    mask[b,c,t] = (t < split_sizes[b,c]) & (c < num_chunks[b]),  t in [0, T-2]
    X = word_embeddings[token_ids_chunk[:, :, :T-1]] * mask      # [B,C,T-1,H]
    logits = (X @ W_dec) @ word_embeddings.T                     # [B,C,T-1,V]
(the gathered decoder positions c+1+t never touch the chunk_units/SOS prefix,
and PAD_VAL == 0, so chunk_units / chunk_sos_embedding cannot affect the output)

Sharding: vocab (tensor-parallel) across the 8 cores; every core processes all
active rows. Masked rows produce exactly-zero logits, so the host compacts the
row set to the unmasked rows (padded to a multiple of 128) and scatters zeros
for the rest.

Per-core device pipeline (all matmuls bf16 with fp32 PSUM accumulation):
  1. dma_gather(transpose=True) pulls the Npad embedding rows out of a bf16
     copy of word_embeddings (+1 zero sentinel row) directly in transposed
     [H-on-partitions] layout.
  2. U^T = W_dec^T @ X^T via PE, PSUM -> SBUF bf16.
  3. logits_shard = U @ E_shard^T via PE, PSUM -> SBUF bf16 -> HBM
     (host upcasts to f32; error stays ~0.4% of absmax, fp32 accumulation).
"""

import numpy as np
import ml_dtypes

B, C, T = 4, 16, 33
TT = T - 1            # 32 token positions actually used
H = 768
HC = H // 128         # 6 contraction chunks
V = 32000
NCORES = 8
VS = V // NCORES      # 4000 vocab columns per core
VT = 500              # vocab tile (one PSUM bank holds 512 f32)
NVT = VS // VT        # 8 vocab tiles
BF16 = ml_dtypes.bfloat16

_KERNELS = {}
last_results = None   # BassKernelResults of the most recent run (for test harness)


def _build(npad: int):
    """Build + compile the 8-core SPMD bass kernel for npad rows (mult of 128)."""
    import concourse.bacc as bacc
    import concourse.bass as bass
    import concourse.mybir as mybir
    import concourse.tile as tile

    dt = mybir.dt
    nc = bacc.Bacc("TRN2", target_bir_lowering=False, debug=False,
                   num_devices=NCORES)

    eaug = nc.dram_tensor("eaug", [V + 1, H], dt.bfloat16, kind="ExternalInput")
    eT = nc.dram_tensor("eT", [128, HC, VS], dt.bfloat16, kind="ExternalInput")
    wd = nc.dram_tensor("wd", [128, HC, H], dt.bfloat16, kind="ExternalInput")
    idx = nc.dram_tensor("idx", [128, npad // 16], dt.int16, kind="ExternalInput")
    # bf16 output (host upcasts): halves the out-DMA bytes; logits are fp32
    # PSUM accumulations so the extra rounding is ~0.2% of absmax
    out = nc.dram_tensor("out", [npad, VS], dt.bfloat16, kind="ExternalOutput")

    # row blocks of <=512 (PSUM bank / moving-free-dim limit)
    blocks = []
    r = 0
    while r < npad:
        s = min(512, npad - r)
        blocks.append((r, s))
        r += s

    with tile.TileContext(nc) as tc:
        with (
            tc.tile_pool(name="const", bufs=1) as cpool,
            tc.tile_pool(name="xt", bufs=1) as xpool,
            tc.tile_pool(name="u", bufs=1) as upool,
            tc.tile_pool(name="outb", bufs=3) as opool,
            tc.tile_pool(name="ps", bufs=8, space=bass.MemorySpace.PSUM) as pspool,
        ):
            # PE warmup: the HAM clock gate holds the PE at 1.2 GHz until it
            # has been busy ~3.4 us. The first ~7 us of the kernel are DMA
            # setup (idx load -> gather -> wd) with an idle PE, so burn that
            # window on junk matmuls; the real matmuls then start at 2.4 GHz.
            warm_sb = cpool.tile([128, 512], dt.bfloat16, tag="warm",
                                 name="warm_sb")
            nc.vector.memset(warm_sb[:], 0.0)
            pw = pspool.tile([128, 512], dt.float32, tag="ps", name="pw")
            for _ in range(16):
                nc.tensor.matmul(pw[:], warm_sb[:, :128], warm_sb[:],
                                 start=True, stop=True)
            # issue order = scheduler priority: idx -> gather -> wd -> eT so
            # the U-phase critical path isn't queued behind the 6 MB eT load
            idx_sb = cpool.tile([128, npad // 16], dt.int16, tag="idx", name="idx_sb")
            nc.sync.dma_start(idx_sb[:], idx.ap()[:])

            from concourse.tile_rust import add_dep_helper
            wd_sb = cpool.tile([128, HC, H], dt.bfloat16, tag="wd", name="wd_sb")
            wd_dma = nc.sync.dma_start(wd_sb[:], wd.ap()[:])

            # 1. gather X^T blocks: xt[p, kc, i] = E[ids[r0+i], kc*128+p]
            xts = []
            gathers = []
            for bi, (r0, sz) in enumerate(blocks):
                xt = xpool.tile([128, HC, sz], dt.bfloat16, tag=f"xt{bi}",
                                name=f"xt{bi}")
                g = nc.gpsimd.dma_gather(
                    xt[:],
                    eaug.ap()[:],
                    idx_sb[:, r0 // 16:(r0 + sz) // 16],
                    sz,
                    sz,
                    H,
                    elem_step=H,
                    transpose=True,
                )
                gathers.append(g)
                xts.append(xt)

            # eT loaded in column halves, all kc of half 0 first: vocab tiles
            # nt<4 then become fully accumulable ~8 us before the full load
            # lands, so PSUM slots recycle and PE stays busy through the tail
            # of the load
            eT_sb = cpool.tile([128, HC, VS], dt.bfloat16, tag="eT", name="eT_sb")
            VH = VS // 2
            last_eT = None
            for half in range(2):
                for kc in range(HC):
                    ev = nc.sync.dma_start(
                        eT_sb[:, kc, half * VH:(half + 1) * VH],
                        eT.ap()[:, kc, half * VH:(half + 1) * VH])
                    # keep the 6 MB eT stream behind the critical-path gather
                    # in the DMA queue
                    add_dep_helper(ev.ins, gathers[-1].ins, sync=True,
                                   reason="eT stream after gathers")
                    last_eT = ev

            # 2. U^T = W_dec^T X^T : u[p, mc, i] = U^T[mc*128+p, r0+i]
            us = []
            for bi, (r0, sz) in enumerate(blocks):
                u = upool.tile([128, HC, sz], dt.bfloat16, tag=f"u{bi}",
                               name=f"u{bi}")
                for mc in range(HC):
                    psu_t = pspool.tile([128, sz], dt.float32, tag="ps",
                                        name="psu_t",
                                        padded_shape=[128, 512])
                    for kc in range(HC):
                        nc.tensor.matmul(
                            psu_t[:],
                            wd_sb[:, kc, mc * 128:(mc + 1) * 128],
                            xts[bi][:, kc, :],
                            start=(kc == 0),
                            stop=(kc == HC - 1),
                        )
                    nc.vector.tensor_copy(u[:, mc, :], psu_t[:])
                us.append(u)

            # 3. logits rows: out[r0+mt*128+p, nt*VT+j]. Accumulation runs in
            # eT-piece arrival order (kc0..kc5) so partial sums proceed while
            # the eT stream is still landing.
            for bi, (r0, sz) in enumerate(blocks):
                for mt in range(sz // 128):
                    ob = opool.tile([128, VS], dt.bfloat16, tag="outb", name="ob")
                    for nt in range(NVT):
                        psl_t = pspool.tile([128, VT], dt.float32, tag="ps",
                                            name="psl_t",
                                            padded_shape=[128, 512])
                        for kc in range(HC):
                            nc.tensor.matmul(
                                psl_t[:],
                                us[bi][:, kc, mt * 128:(mt + 1) * 128],
                                eT_sb[:, kc, nt * VT:(nt + 1) * VT],
                                start=(kc == 0),
                                stop=(kc == HC - 1),
                            )
                        nc.vector.tensor_copy(ob[:, nt * VT:(nt + 1) * VT],
                                              psl_t[:])
                        # the last m-tile streams out in quarters so the final
                        # (critical-path) DMA piece is small
                        last_tile = (bi == len(blocks) - 1
                                     and mt == sz // 128 - 1)
                        piece = 2 if last_tile else 4
                        if (nt + 1) % piece == 0:
                            h0 = (nt + 1 - piece) * VT
                            od = nc.sync.dma_start(
                                out.ap()[r0 + mt * 128:r0 + (mt + 1) * 128,
                                         h0:h0 + piece * VT],
                                ob[:, h0:h0 + piece * VT])
                            # out DMAs are never the critical path until the
                            # very end; keep them behind the eT stream so PE
                            # isn't starved of eT pieces mid-kernel
                            add_dep_helper(od.ins, last_eT.ins, sync=True,
                                           reason="out DMAs after eT stream")

    nc.compile()
    return nc


def _get_kernel(npad: int):
    if npad not in _KERNELS:
        _KERNELS[npad] = _build(npad)
    return _KERNELS[npad]


def prep_inputs(token_ids, split_sizes, num_chunks, E, Wd):
    """Host-side shard prep. Returns (in_maps, rows, npad) or (None, rows, 0)."""
    b, c, t = token_ids.shape
    tt = t - 1
    mask = ((np.arange(tt)[None, None, :] < split_sizes[:, :, None])
            & (np.arange(c)[None, :, None] < num_chunks[:, None, None]))
    flat_ids = token_ids[:, :, :tt].reshape(-1).astype(np.int64)
    rows = np.nonzero(mask.reshape(-1))[0]
    nact = len(rows)
    if nact == 0:
        return None, rows, 0
    npad = ((nact + 127) // 128) * 128
    ids_c = np.full(npad, V, dtype=np.int64)     # sentinel -> zero row
    ids_c[:nact] = flat_ids[rows]
    # wrapped in 16 partitions; HW SWDGE requires the block replicated across
    # all 8 Q7 partition groups (the simulator reads only the first 16 rows)
    idx_np = np.tile(ids_c.reshape(npad // 16, 16).T.astype(np.int16), (8, 1))

    Ebf = E.astype(BF16)
    eaug_np = np.zeros((V + 1, H), BF16)
    eaug_np[:V] = Ebf
    wd_np = np.ascontiguousarray(
        Wd.astype(BF16).reshape(HC, 128, H).transpose(1, 0, 2))
    in_maps = []
    for k in range(NCORES):
        eT_np = np.ascontiguousarray(
            Ebf[k * VS:(k + 1) * VS].reshape(VS, HC, 128).transpose(2, 1, 0))
        in_maps.append({"eaug": eaug_np, "eT": eT_np, "wd": wd_np,
                        "idx": idx_np})
    return in_maps, rows, npad


def kernel(**inputs) -> np.ndarray:
    global last_results
    token_ids = np.asarray(inputs["token_ids_chunk"])
    split_sizes = np.asarray(inputs["split_sizes"])
    num_chunks = np.asarray(inputs["num_chunks"])
    E = np.asarray(inputs["word_embeddings"], dtype=np.float32)
    Wd = np.asarray(inputs["W_dec"], dtype=np.float32)
    # chunk_units / chunk_sos_embedding provably do not affect the output.

    b, c, t = token_ids.shape
    tt = t - 1
    outF = np.zeros((b * c * tt, V), dtype=np.float32)

    in_maps, rows, npad = prep_inputs(token_ids, split_sizes, num_chunks, E, Wd)
    if in_maps is not None:
        from concourse import bass_utils
        nc = _get_kernel(npad)
        res = bass_utils.run_bass_kernel_spmd(
            nc, in_maps, core_ids=list(range(NCORES)))
        last_results = res
        nact = len(rows)
        shard = np.concatenate(
            [res.results[k]["out"][:nact].astype(np.float32)
             for k in range(NCORES)], axis=1)
        outF[rows] = shard
    return outF.reshape(b, c, tt, V)

